# revision 11
# baseline (speedup 1.0000x reference)
"""Trainium2 Bass kernel for AdvancedNeuralMemory (gated linear recurrence memory).

Math (see reference):
  h  = x @ Wd^T + bd
  q  = LN(h @ Wq^T + bq) * qn_g + qn_b
  k  = LN(h @ Wk^T + bk) * kn_g + kn_b
  v  = h @ Wv^T + bv
  mem_t = gate * mem_{t-1} + lr * k_t * v_t      (gate = 1 - forget_factor, scalar)
  out = (q * mem) @ Wu^T + bu

Key facts exploited:
  * gate is a scalar constant -> the scan over a 128-token tile is a lower
    triangular Toeplitz matmul L @ (k*v) plus a rank-1 carry term, and the
    carry decays like gate^k, so splitting the sequence across cores only
    needs a short (256 token) recompute warmup instead of communication.
  * Sharding: 8 cores = 4 batch elements x 2 sequence halves.

Layouts (host-prepared so every DMA is contiguous and matmul-ready):
  xt  [NB, 128, 1024] bf16 : xt[b, p, dc*128+t] = x[token 128b+t, d=128dc+p]
                             (i.e. x transposed into 128x128 tiles, d on partitions)
  wd  [128, 8, 512]   bf16 : wd[p, dc, m] = Wd[m, 128dc+p]         (lhsT tiles)
  wq/wk/wv [128, 4, 512] bf16 : wq[p, mc, n] = Wq[n, 128mc+p]      (rhs tiles)
  wu  [128, 4, 1024]  bf16 : wu[p, jc, d] = Wu'[d, 128jc+p]        (rhs tiles)
  lt  [128, 128] : lt[i, t] = lr * gate^(t-i) (t >= i else 0)
  gp  [1, 128]   : gp[0, t] = gate^(t+1)
  wcol[128, 1]   : wcol[i, 0] = lr * gate^(127-i)
  g128[1, 1]     : gate^128
"""

import os
import sys
from contextlib import ExitStack

import numpy as np

for _p in ("/opt/trn_rl_repo",):
    if os.path.isdir(_p) and _p not in sys.path:
        sys.path.insert(0, _p)

import ml_dtypes  # noqa: E402


def _ensure_axon_hooks_stub():
    """concourse's axon trace path imports antenv.axon_hooks, which this
    image lacks; provide a stub so tracing degrades instead of crashing."""
    import types

    try:
        import antenv.axon_hooks  # noqa: F401
        return
    except ImportError:
        pass
    try:
        import antenv
    except ImportError:
        return
    mod = types.ModuleType("antenv.axon_hooks")
    mod._hook = None
    mod.set_axon_ntff_profile_hook = lambda h: setattr(mod, "_hook", h)
    mod.get_axon_ntff_profile_hook = lambda: mod._hook
    sys.modules["antenv.axon_hooks"] = mod
    antenv.axon_hooks = mod


_ensure_axon_hooks_stub()

import concourse.bacc as bacc  # noqa: E402
import concourse.tile as tile  # noqa: E402
from concourse import mybir  # noqa: E402
from concourse import bass_utils  # noqa: E402

BF16 = ml_dtypes.bfloat16
P = 128
D = 1024
DM = 512
DC = D // P   # 8 d-chunks
MC = DM // P  # 4 m/n-chunks
EPS = 1e-5

# full-problem config
B_FULL, S_FULL = 4, 8192
N_CORES = 8
WARM_FULL = 1                     # warmup blocks (128 tokens); gate^128 ~ 1.4e-6
NOUT_FULL = S_FULL // 2 // P      # 32 output blocks per core
NB_FULL = WARM_FULL + NOUT_FULL   # 34

_LAST_RESULTS = None  # BassKernelResults of the last kernel() call (for test.py)


def build_nc(NB, WARM, n_cores, g128_f, use_bias=False, use_nb=False):
    """Build + compile the Bass program.

    use_bias: include bd/bq/bk/bv/bu bias-add instructions (K=1 matmul rows).
    use_nb:   include qn/kn affine (gain+bias) applied explicitly on q/k
              (otherwise gains are folded into wv/wu host-side and biases
              are assumed zero).
    """
    dt = mybir.dt
    f32, bf16 = dt.float32, dt.bfloat16
    NOUT = NB - WARM

    nc = bacc.Bacc(
        "TRN2", target_bir_lowering=False, debug=False, num_devices=n_cores
    )

    def inp(name, shape, dty=bf16):
        return nc.dram_tensor(name, shape, dty, kind="ExternalInput").ap()

    xt = inp("xt", [NB, P, D])
    wd = inp("wd", [P, DC, DM])
    wq = inp("wq", [P, MC, DM])
    wk = inp("wk", [P, MC, DM])
    wv = inp("wv", [P, MC, DM])
    wu = inp("wu", [P, MC, D])
    lt = inp("lt", [P, P])
    gp = inp("gp", [1, P])
    wcol = inp("wcol", [P, 1])
    ident = inp("ident", [P, P])
    if use_bias:
        bd = inp("bd", [1, DM])
        bq = inp("bq", [1, DM])
        bk = inp("bk", [1, DM])
        bv = inp("bv", [1, DM])
        bu = inp("bu", [1, D])
    if use_nb:
        qng = inp("qng", [P, DM])
        qnb = inp("qnb", [P, DM])
        kng = inp("kng", [P, DM])
        knb = inp("knb", [P, DM])

    out = nc.dram_tensor("out", [NOUT * P, D], f32, kind="ExternalOutput").ap()

    AF = mybir.ActivationFunctionType
    OP = mybir.AluOpType

    with tile.TileContext(nc) as tc, ExitStack() as ctx:
        consts = ctx.enter_context(tc.tile_pool(name="consts", bufs=1))
        sb_in = ctx.enter_context(tc.tile_pool(name="sb_in", bufs=4))
        sb_mid = ctx.enter_context(tc.tile_pool(name="sb_mid", bufs=3))
        sb_out = ctx.enter_context(tc.tile_pool(name="sb_out", bufs=3))
        sb_carry = ctx.enter_context(tc.tile_pool(name="sb_carry", bufs=3))
        stats = ctx.enter_context(tc.tile_pool(name="stats", bufs=4))
        ps_hT = ctx.enter_context(tc.tile_pool(name="ps_hT", bufs=1, space="PSUM"))
        ps_big = ctx.enter_context(tc.tile_pool(name="ps_big", bufs=3, space="PSUM"))
        ps_mem = ctx.enter_context(tc.tile_pool(name="ps_mem", bufs=1, space="PSUM"))
        ps_cr = ctx.enter_context(tc.tile_pool(name="ps_cr", bufs=1, space="PSUM"))
        ps_rT = ctx.enter_context(tc.tile_pool(name="ps_rT", bufs=1, space="PSUM"))
        ps_out = ctx.enter_context(tc.tile_pool(name="ps_out", bufs=1, space="PSUM"))

        def load_const(ap_in, shape, dty=bf16):
            t = consts.tile(shape, dty, tag=ap_in.tensor.name + "_sb")
            nc.sync.dma_start(out=t, in_=ap_in)
            return t

        # order matters: wd + scan consts first so the first block's down-proj
        # can start while the larger qkv/up weights are still loading
        wd_sb = load_const(wd, [P, DC, DM])
        lt_sb = load_const(lt, [P, P])
        gp_sb = load_const(gp, [1, P])
        wcol_sb = load_const(wcol, [P, 1])
        ident_sb = load_const(ident, [P, P])
        wk_sb = load_const(wk, [P, MC, DM])
        wv_sb = load_const(wv, [P, MC, DM])
        wq_sb = load_const(wq, [P, MC, DM])
        wu_sb = load_const(wu, [P, MC, D])
        if use_bias:
            bd_sb = load_const(bd, [1, DM])
            bq_sb = load_const(bq, [1, DM])
            bk_sb = load_const(bk, [1, DM])
            bv_sb = load_const(bv, [1, DM])
            bu_sb = load_const(bu, [1, D])
            ones_sb = consts.tile([1, P], bf16, tag="ones")
            nc.vector.memset(ones_sb, 1.0)
        if use_nb:
            qng_sb = load_const(qng, [P, DM])
            qnb_sb = load_const(qnb, [P, DM])
            kng_sb = load_const(kng, [P, DM])
            knb_sb = load_const(knb, [P, DM])

        eps_sb = consts.tile([P, 1], f32, tag="eps")
        nc.vector.memset(eps_sb, EPS)
        zcarry = consts.tile([1, DM], bf16, tag="zcarry")
        nc.vector.memset(zcarry, 0.0)

        def layernorm(src_ps, dst_dtype, tag, gtile=None, btile=None):
            """LN along free axis of [P, DM] PSUM tile -> SBUF tile."""
            st = stats.tile([P, 6], f32, tag=f"st_{tag}")
            nc.vector.bn_stats(out=st, in_=src_ps)
            mv = stats.tile([P, 2], f32, tag=f"mv_{tag}")
            nc.vector.bn_aggr(out=mv, in_=st)
            sd = stats.tile([P, 1], f32, tag=f"sd_{tag}")
            nc.scalar.activation(
                out=sd, in_=mv[:, 1:2], func=AF.Sqrt, bias=eps_sb, scale=1.0
            )
            rstd = stats.tile([P, 1], f32, tag=f"rstd_{tag}")
            nc.vector.reciprocal(out=rstd, in_=sd)
            dst = sb_mid.tile([P, DM], dst_dtype, tag=f"ln_{tag}")
            nc.vector.tensor_scalar(
                out=dst,
                in0=src_ps,
                scalar1=mv[:, 0:1],
                scalar2=rstd,
                op0=OP.subtract,
                op1=OP.mult,
            )
            if gtile is not None:
                nc.vector.tensor_tensor(dst, dst, gtile, OP.mult)
            if btile is not None:
                nc.vector.tensor_tensor(dst, dst, btile, OP.add)
            return dst

        # Three-stage software pipeline: block b's projections interleave with
        # block b-1's scan/retrieval (mem, r) and block b-2's transpose/
        # up-proj/store, so the in-order PE never waits on the ACT hT copy or
        # the DVE-produced r tile, and PSUM stays within 8 banks.
        def stage2_mem(pv):
            """Scan matmuls for a previous block (PE, cheap, emitted early)."""
            mem_ps = ps_mem.tile([P, DM], f32, tag="mem")
            nc.tensor.matmul(mem_ps, lhsT=lt_sb, rhs=pv["u"], start=True, stop=False)
            nc.tensor.matmul(
                mem_ps, lhsT=gp_sb, rhs=pv["carry_in"], start=False, stop=True
            )
            pv["mem"] = mem_ps

        def stage2_r(pv):
            r_sb = sb_mid.tile([P, DM], bf16, tag="r")
            nc.vector.tensor_tensor(r_sb, pv["q"], pv["mem"], OP.mult)
            pv["r"] = r_sb

        def stage2_up(pv):
            rT_ps = ps_rT.tile([P, MC, P], bf16, tag="rT")
            for j in range(MC):
                nc.tensor.transpose(
                    rT_ps[:, j], pv["r"][:, j * P : (j + 1) * P], ident_sb
                )
            rT_sb = sb_mid.tile([P, MC, P], bf16, tag="rT_sb")
            nc.scalar.copy(out=rT_sb, in_=rT_ps)

            o_sb = sb_out.tile([P, D], f32, tag="osb")
            for h in range(2):
                o_ps = ps_out.tile([P, DM], f32, tag="ops")
                for j in range(MC):
                    nc.tensor.matmul(
                        o_ps,
                        lhsT=rT_sb[:, j],
                        rhs=wu_sb[:, j, h * DM : (h + 1) * DM],
                        start=(j == 0),
                        stop=(j == MC - 1) and not use_bias,
                    )
                if use_bias:
                    nc.tensor.matmul(
                        o_ps, lhsT=ones_sb, rhs=bu_sb[:, h * DM : (h + 1) * DM],
                        start=False, stop=True,
                    )
                nc.scalar.copy(out=o_sb[:, h * DM : (h + 1) * DM], in_=o_ps)
            nc.sync.dma_start(
                out=out[pv["oidx"] * P : (pv["oidx"] + 1) * P, :], in_=o_sb
            )

        carry_prev = zcarry
        prev = None
        prev2 = None
        for b in range(NB):
            is_out = b >= WARM

            xt_sb = sb_in.tile([P, D], bf16, tag="xt")
            nc.sync.dma_start(out=xt_sb, in_=xt[b])

            # ---- down-projection: hT[m, t] (4 chunks of 128 m) ----
            hT_ps = ps_hT.tile([P, MC, P], f32, tag="hT")
            for mc in range(MC):
                for dc in range(DC):
                    nc.tensor.matmul(
                        hT_ps[:, mc, :],
                        lhsT=wd_sb[:, dc, mc * P : (mc + 1) * P],
                        rhs=xt_sb[:, dc * P : (dc + 1) * P],
                        start=(dc == 0),
                        stop=(dc == DC - 1) if not use_bias else False,
                    )
                if use_bias:
                    nc.tensor.matmul(
                        hT_ps[:, mc, :],
                        lhsT=bd_sb[:, mc * P : (mc + 1) * P],
                        rhs=ones_sb,
                        start=False,
                        stop=True,
                    )
            hT_sb = sb_mid.tile([P, MC, P], bf16, tag="hT_sb")
            nc.scalar.copy(out=hT_sb, in_=hT_ps)

            # block b-2's transpose/up-proj/store: PE work that hides the
            # ACT hT copy latency before qkv needs hT_sb
            if prev2 is not None:
                stage2_up(prev2)

            # ---- k/v (+q for output blocks) projections ----
            k_ps = ps_big.tile([P, DM], f32, tag="qkv")
            v_ps = ps_big.tile([P, DM], f32, tag="qkv")
            if is_out:
                q_ps = ps_big.tile([P, DM], f32, tag="qkv")
            for mc in range(MC):
                last = mc == MC - 1
                nc.tensor.matmul(
                    k_ps, lhsT=hT_sb[:, mc], rhs=wk_sb[:, mc],
                    start=(mc == 0), stop=last and not use_bias,
                )
                nc.tensor.matmul(
                    v_ps, lhsT=hT_sb[:, mc], rhs=wv_sb[:, mc],
                    start=(mc == 0), stop=last and not use_bias,
                )
                if is_out:
                    nc.tensor.matmul(
                        q_ps, lhsT=hT_sb[:, mc], rhs=wq_sb[:, mc],
                        start=(mc == 0), stop=last and not use_bias,
                    )
            if use_bias:
                nc.tensor.matmul(k_ps, lhsT=ones_sb, rhs=bk_sb, start=False, stop=True)
                nc.tensor.matmul(v_ps, lhsT=ones_sb, rhs=bv_sb, start=False, stop=True)
                if is_out:
                    nc.tensor.matmul(
                        q_ps, lhsT=ones_sb, rhs=bq_sb, start=False, stop=True
                    )

            # previous block's scan matmuls (PE) + r (DVE): r overlaps with
            # this block's LN/carry work and is ready before next iteration's
            # transposes
            if prev is not None:
                stage2_mem(prev)
                stage2_r(prev)

            # ---- layernorms + u = k*v ----
            k_sb = layernorm(
                k_ps, bf16, "k",
                gtile=kng_sb if use_nb else None,
                btile=knb_sb if use_nb else None,
            )
            u_sb = sb_mid.tile([P, DM], bf16, tag="u")
            nc.vector.tensor_tensor(u_sb, k_sb, v_ps, OP.mult)

            # ---- carry update: carry' = wcol^T @ u + gate^128 * carry ----
            # the gate^128*carry term + PSUM eviction fold into one DVE op
            c_ps = ps_cr.tile([1, DM], f32, tag="cps")
            nc.tensor.matmul(c_ps, lhsT=wcol_sb, rhs=u_sb, start=True, stop=True)
            carry_new = sb_carry.tile([1, DM], bf16, tag="carry")
            nc.vector.scalar_tensor_tensor(
                out=carry_new, in0=carry_prev, scalar=g128_f, in1=c_ps,
                op0=OP.mult, op1=OP.add,
            )

            if is_out:
                q_sb = layernorm(
                    q_ps, bf16, "q",
                    gtile=qng_sb if use_nb else None,
                    btile=qnb_sb if use_nb else None,
                )
                prev2 = prev
                prev = {
                    "q": q_sb,
                    "u": u_sb,
                    "carry_in": carry_prev,
                    "oidx": b - WARM,
                }
            else:
                prev2 = prev
                prev = None

            carry_prev = carry_new

        if prev is not None:
            stage2_mem(prev)
            stage2_r(prev)
        if prev2 is not None:
            stage2_up(prev2)
        if prev is not None:
            stage2_up(prev)

    nc.compile()
    return nc


def prep_weights(Wd, bd, Wq, bq, Wk, bk, Wv, bv, Wu, bu,
                 qn_g, qn_b, kn_g, kn_b, adaptive_lr, forget_factor):
    """Host-side packing of all weight-derived inputs. Returns (in_map, flags)."""
    gate = float(1.0 - np.float64(forget_factor.reshape(-1)[0]))
    lr = float(np.float64(adaptive_lr.reshape(-1)[0]))

    use_bias = any(np.any(np.asarray(a) != 0) for a in (bd, bq, bk, bv, bu))
    use_nb = bool(np.any(np.asarray(qn_b) != 0) or np.any(np.asarray(kn_b) != 0))

    Wd = np.asarray(Wd, np.float32)
    Wq = np.asarray(Wq, np.float32)
    Wk = np.asarray(Wk, np.float32)
    Wv = np.asarray(Wv, np.float32).copy()
    Wu = np.asarray(Wu, np.float32).copy()
    bv_eff = np.asarray(bv, np.float32).copy()
    qn_g = np.asarray(qn_g, np.float32)
    kn_g = np.asarray(kn_g, np.float32)

    if not use_nb:
        # fold gains: k*g into v's weights (u = k*(g*v)), q*g into Wu rows
        Wv *= kn_g[:, None]
        bv_eff *= kn_g
        Wu *= qn_g[None, :]

    def pack_rhs(W, nchunks):  # W [N, K] -> [P, nchunks, N] with K on partitions
        A = np.ascontiguousarray(W.T)  # [K, N]
        return np.ascontiguousarray(
            A.reshape(nchunks, P, A.shape[1]).transpose(1, 0, 2)
        ).astype(BF16)

    wd_h = pack_rhs(Wd, DC)   # [128, 8, 512]  (used as lhsT tiles)
    wq_h = pack_rhs(Wq, MC)
    wk_h = pack_rhs(Wk, MC)
    wv_h = pack_rhs(Wv, MC)
    wu_h = pack_rhs(Wu, MC)   # [128, 4, 1024]

    ii = np.arange(P, dtype=np.float64)
    tt = ii[None, :]
    dd = tt - ii[:, None]  # t - i
    lt_h = np.where(dd >= 0, lr * gate ** np.maximum(dd, 0), 0.0).astype(BF16)
    gp_h = (gate ** (ii + 1.0)).reshape(1, P).astype(BF16)
    wcol_h = (lr * gate ** (127.0 - ii)).reshape(P, 1).astype(BF16)
    ident_h = np.eye(P, dtype=np.float32).astype(BF16)

    in_map = {
        "wd": wd_h, "wq": wq_h, "wk": wk_h, "wv": wv_h, "wu": wu_h,
        "lt": lt_h, "gp": gp_h, "wcol": wcol_h, "ident": ident_h,
    }
    if use_bias:
        in_map["bd"] = np.asarray(bd, np.float32).reshape(1, DM).astype(BF16)
        in_map["bq"] = np.asarray(bq, np.float32).reshape(1, DM).astype(BF16)
        in_map["bk"] = np.asarray(bk, np.float32).reshape(1, DM).astype(BF16)
        in_map["bv"] = bv_eff.reshape(1, DM).astype(BF16)
        in_map["bu"] = np.asarray(bu, np.float32).reshape(1, D).astype(BF16)
    if use_nb:
        in_map["qng"] = np.broadcast_to(qn_g, (P, DM)).astype(BF16)
        in_map["qnb"] = np.broadcast_to(np.asarray(qn_b, np.float32), (P, DM)).astype(BF16)
        in_map["kng"] = np.broadcast_to(kn_g, (P, DM)).astype(BF16)
        in_map["knb"] = np.broadcast_to(np.asarray(kn_b, np.float32), (P, DM)).astype(BF16)
    return in_map, use_bias, use_nb, float(gate ** 128.0)


def pack_x_tokens(xs):
    """xs [T, D] f32 (T multiple of 128) -> [T//128, 128, 1024] bf16 tiled/transposed."""
    T = xs.shape[0]
    nb = T // P
    return np.ascontiguousarray(
        xs.reshape(nb, P, DC, P).transpose(0, 3, 2, 1).reshape(nb, P, D)
    ).astype(BF16)


_NC_CACHE = {}


def _get_nc(NB, WARM, n_cores, g128_f, use_bias, use_nb):
    key = (NB, WARM, n_cores, g128_f, use_bias, use_nb)
    if key not in _NC_CACHE:
        _NC_CACHE[key] = build_nc(NB, WARM, n_cores, g128_f, use_bias, use_nb)
    return _NC_CACHE[key]


def kernel(x, Wd, bd, Wq, bq, Wk, bk, Wv, bv, Wu, bu,
           qn_g, qn_b, kn_g, kn_b, adaptive_lr, forget_factor):
    global _LAST_RESULTS
    x = np.asarray(x, np.float32)
    assert x.shape == (B_FULL, S_FULL, D), x.shape

    w_map, use_bias, use_nb, g128_f = prep_weights(
        Wd, bd, Wq, bq, Wk, bk, Wv, bv, Wu, bu,
        qn_g, qn_b, kn_g, kn_b, adaptive_lr, forget_factor,
    )

    nc = _get_nc(NB_FULL, WARM_FULL, N_CORES, g128_f, use_bias, use_nb)

    half_len = S_FULL // 2  # 4096
    warm = WARM_FULL * P    # 256
    in_maps = []
    for c in range(N_CORES):
        bidx, half = divmod(c, 2)
        if half == 0:
            xs = np.zeros((warm + half_len, D), np.float32)
            xs[warm:] = x[bidx, :half_len]
        else:
            xs = x[bidx, half_len - warm :]
        m = dict(w_map)
        m["xt"] = pack_x_tokens(xs)
        in_maps.append(m)

    trace = bool(os.environ.get("KERNEL_TRACE"))
    res = bass_utils.run_bass_kernel_spmd(
        nc, in_maps, core_ids=list(range(N_CORES)), trace=trace
    )
    _LAST_RESULTS = res

    out = np.empty((B_FULL, S_FULL, D), np.float32)
    for c in range(N_CORES):
        bidx, half = divmod(c, 2)
        out[bidx, half * half_len : (half + 1) * half_len] = res.results[c]["out"]
    return out


# revision 13
# speedup vs baseline: 1.1406x; 1.1406x over previous
"""Trainium2 Bass kernel for AdvancedNeuralMemory (gated linear recurrence memory).

Math (see reference):
  h  = x @ Wd^T + bd
  q  = LN(h @ Wq^T + bq) * qn_g + qn_b
  k  = LN(h @ Wk^T + bk) * kn_g + kn_b
  v  = h @ Wv^T + bv
  mem_t = gate * mem_{t-1} + lr * k_t * v_t      (gate = 1 - forget_factor, scalar)
  out = (q * mem) @ Wu^T + bu

Key facts exploited:
  * gate is a scalar constant -> the scan over a 128-token tile is a lower
    triangular Toeplitz matmul L @ (k*v) plus a rank-1 carry term, and the
    carry decays like gate^k, so splitting the sequence across cores only
    needs a short (256 token) recompute warmup instead of communication.
  * Sharding: 8 cores = 4 batch elements x 2 sequence halves.

Layouts (host-prepared so every DMA is contiguous and matmul-ready):
  xt  [NB, 128, 1024] bf16 : xt[b, p, dc*128+t] = x[token 128b+t, d=128dc+p]
                             (i.e. x transposed into 128x128 tiles, d on partitions)
  wd  [128, 8, 512]   bf16 : wd[p, dc, m] = Wd[m, 128dc+p]         (lhsT tiles)
  wq/wk/wv [128, 4, 512] bf16 : wq[p, mc, n] = Wq[n, 128mc+p]      (rhs tiles)
  wu  [128, 4, 1024]  bf16 : wu[p, jc, d] = Wu'[d, 128jc+p]        (rhs tiles)
  lt  [128, 128] : lt[i, t] = lr * gate^(t-i) (t >= i else 0)

Carry handling: the incoming carry is folded into row 0 of the u tile
(u'[0] = k0*v0 + (gate/lr)*carry) so mem = LT @ u' is a single matmul and the
outgoing carry is the single matmul wcol^T @ u' (wcol[i] = lr*gate^(127-i),
whose i=0 term reproduces the gate^128 decay of the incoming carry).
"""

import os
import sys
from contextlib import ExitStack

import numpy as np

for _p in ("/opt/trn_rl_repo",):
    if os.path.isdir(_p) and _p not in sys.path:
        sys.path.insert(0, _p)

import ml_dtypes  # noqa: E402


def _ensure_axon_hooks_stub():
    """concourse's axon trace path imports antenv.axon_hooks, which this
    image lacks; provide a stub so tracing degrades instead of crashing."""
    import types

    try:
        import antenv.axon_hooks  # noqa: F401
        return
    except ImportError:
        pass
    try:
        import antenv
    except ImportError:
        return
    mod = types.ModuleType("antenv.axon_hooks")
    mod._hook = None
    mod.set_axon_ntff_profile_hook = lambda h: setattr(mod, "_hook", h)
    mod.get_axon_ntff_profile_hook = lambda: mod._hook
    sys.modules["antenv.axon_hooks"] = mod
    antenv.axon_hooks = mod


_ensure_axon_hooks_stub()

import concourse.bacc as bacc  # noqa: E402
import concourse.tile as tile  # noqa: E402
from concourse import mybir  # noqa: E402
from concourse import bass_utils  # noqa: E402

BF16 = ml_dtypes.bfloat16
P = 128
D = 1024
DM = 512
DC = D // P   # 8 d-chunks
MC = DM // P  # 4 m/n-chunks
EPS = 1e-5

# full-problem config
B_FULL, S_FULL = 4, 8192
N_CORES = 8
WARM_FULL = 1                     # warmup blocks (128 tokens); gate^128 ~ 1.4e-6
NOUT_FULL = S_FULL // 2 // P      # 32 output blocks per core
NB_FULL = WARM_FULL + NOUT_FULL   # 34

_LAST_RESULTS = None  # BassKernelResults of the last kernel() call (for test.py)


def build_nc(NB, WARM, n_cores, gol_f, use_bias=False, use_nb=False):
    """Build + compile the Bass program.

    use_bias: include bd/bq/bk/bv/bu bias-add instructions (K=1 matmul rows).
    use_nb:   include qn/kn affine (gain+bias) applied explicitly on q/k
              (otherwise gains are folded into wv/wu host-side and biases
              are assumed zero).
    """
    dt = mybir.dt
    f32, bf16 = dt.float32, dt.bfloat16
    NOUT = NB - WARM

    nc = bacc.Bacc(
        "TRN2", target_bir_lowering=False, debug=False, num_devices=n_cores
    )

    def inp(name, shape, dty=bf16):
        return nc.dram_tensor(name, shape, dty, kind="ExternalInput").ap()

    xt = inp("xt", [NB, P, D])
    wd = inp("wd", [P, DC, DM])
    wq = inp("wq", [P, MC, DM])
    wk = inp("wk", [P, MC, DM])
    wv = inp("wv", [P, MC, DM])
    wu = inp("wu", [P, MC, D])
    lt = inp("lt", [P, P])
    wcol = inp("wcol", [P, 1])
    ident = inp("ident", [P, P])
    if use_bias:
        bd = inp("bd", [1, DM])
        bq = inp("bq", [1, DM])
        bk = inp("bk", [1, DM])
        bv = inp("bv", [1, DM])
        bu = inp("bu", [1, D])
    if use_nb:
        qng = inp("qng", [P, DM])
        qnb = inp("qnb", [P, DM])
        kng = inp("kng", [P, DM])
        knb = inp("knb", [P, DM])

    out = nc.dram_tensor("out", [NOUT * P, D], f32, kind="ExternalOutput").ap()

    AF = mybir.ActivationFunctionType
    OP = mybir.AluOpType

    with tile.TileContext(nc) as tc, ExitStack() as ctx:
        consts = ctx.enter_context(tc.tile_pool(name="consts", bufs=1))
        sb_in = ctx.enter_context(tc.tile_pool(name="sb_in", bufs=4))
        sb_mid = ctx.enter_context(tc.tile_pool(name="sb_mid", bufs=3))
        sb_out = ctx.enter_context(tc.tile_pool(name="sb_out", bufs=3))
        stats = ctx.enter_context(tc.tile_pool(name="stats", bufs=4))
        ps_hT = ctx.enter_context(tc.tile_pool(name="ps_hT", bufs=1, space="PSUM"))
        ps_big = ctx.enter_context(tc.tile_pool(name="ps_big", bufs=3, space="PSUM"))
        ps_mem = ctx.enter_context(tc.tile_pool(name="ps_mem", bufs=1, space="PSUM"))
        ps_cr = ctx.enter_context(tc.tile_pool(name="ps_cr", bufs=1, space="PSUM"))
        ps_rT = ctx.enter_context(tc.tile_pool(name="ps_rT", bufs=1, space="PSUM"))
        ps_out = ctx.enter_context(tc.tile_pool(name="ps_out", bufs=1, space="PSUM"))

        def load_const(ap_in, shape, dty=bf16):
            t = consts.tile(shape, dty, tag=ap_in.tensor.name + "_sb")
            nc.sync.dma_start(out=t, in_=ap_in)
            return t

        # order matters: wd + scan consts first so the first block's down-proj
        # can start while the larger qkv/up weights are still loading
        wd_sb = load_const(wd, [P, DC, DM])
        lt_sb = load_const(lt, [P, P])
        wcol_sb = load_const(wcol, [P, 1])
        ident_sb = load_const(ident, [P, P])
        wk_sb = load_const(wk, [P, MC, DM])
        wv_sb = load_const(wv, [P, MC, DM])
        wq_sb = load_const(wq, [P, MC, DM])
        wu_sb = load_const(wu, [P, MC, D])
        if use_bias:
            bd_sb = load_const(bd, [1, DM])
            bq_sb = load_const(bq, [1, DM])
            bk_sb = load_const(bk, [1, DM])
            bv_sb = load_const(bv, [1, DM])
            bu_sb = load_const(bu, [1, D])
            ones_sb = consts.tile([1, P], bf16, tag="ones")
            nc.vector.memset(ones_sb, 1.0)
        if use_nb:
            qng_sb = load_const(qng, [P, DM])
            qnb_sb = load_const(qnb, [P, DM])
            kng_sb = load_const(kng, [P, DM])
            knb_sb = load_const(knb, [P, DM])

        eps_sb = consts.tile([P, 1], f32, tag="eps")
        nc.vector.memset(eps_sb, EPS)

        def layernorm(src_ps, dst_dtype, tag, gtile=None, btile=None):
            """LN along free axis of [P, DM] PSUM tile -> SBUF tile."""
            st = stats.tile([P, 6], f32, tag=f"st_{tag}")
            nc.vector.bn_stats(out=st, in_=src_ps)
            mv = stats.tile([P, 2], f32, tag=f"mv_{tag}")
            nc.vector.bn_aggr(out=mv, in_=st)
            sd = stats.tile([P, 1], f32, tag=f"sd_{tag}")
            nc.scalar.activation(
                out=sd, in_=mv[:, 1:2], func=AF.Sqrt, bias=eps_sb, scale=1.0
            )
            rstd = stats.tile([P, 1], f32, tag=f"rstd_{tag}")
            nc.vector.reciprocal(out=rstd, in_=sd)
            dst = sb_mid.tile([P, DM], dst_dtype, tag=f"ln_{tag}")
            nc.vector.tensor_scalar(
                out=dst,
                in0=src_ps,
                scalar1=mv[:, 0:1],
                scalar2=rstd,
                op0=OP.subtract,
                op1=OP.mult,
            )
            if gtile is not None:
                nc.vector.tensor_tensor(dst, dst, gtile, OP.mult)
            if btile is not None:
                nc.vector.tensor_tensor(dst, dst, btile, OP.add)
            return dst

        # Software pipeline (3 blocks deep). Per iteration b the PE stream is
        #   down(b) | up-proj(b-2) | mem(b-1) | qkv(b) | transpose(b-1)
        # so every PE -> ACT/DVE -> PE round trip (hT copy, r = q*mem, rT
        # copy) is covered by >=1.7us of independent PE work. The scan carry
        # is folded into u row 0 by a DVE op, and extracted by a single
        # wcol matmul (engines cannot read a lone partition 127, so the
        # mem[127,:] shortcut is not available).
        def stage2_mem(pv):
            mem_ps = ps_mem.tile([P, DM], f32, tag="mem")
            nc.tensor.matmul(mem_ps, lhsT=lt_sb, rhs=pv["u"], start=True, stop=True)
            pv["mem"] = mem_ps

        def stage2_r(pv):
            r_sb = sb_mid.tile([P, DM], bf16, tag="r")
            nc.vector.tensor_tensor(r_sb, pv["q"], pv["mem"], OP.mult)
            pv["r"] = r_sb

        def stage2_transp(pv):
            rT_ps = ps_rT.tile([P, MC, P], bf16, tag="rT")
            for j in range(MC):
                nc.tensor.transpose(
                    rT_ps[:, j], pv["r"][:, j * P : (j + 1) * P], ident_sb
                )
            rT_sb = sb_mid.tile([P, MC, P], bf16, tag="rT_sb")
            nc.scalar.copy(out=rT_sb, in_=rT_ps)
            pv["rT"] = rT_sb

        def stage2_upmm(pv):
            o_sb = sb_out.tile([P, D], f32, tag="osb")
            for h in range(2):
                o_ps = ps_out.tile([P, DM], f32, tag="ops")
                for j in range(MC):
                    nc.tensor.matmul(
                        o_ps,
                        lhsT=pv["rT"][:, j],
                        rhs=wu_sb[:, j, h * DM : (h + 1) * DM],
                        start=(j == 0),
                        stop=(j == MC - 1) and not use_bias,
                    )
                if use_bias:
                    nc.tensor.matmul(
                        o_ps, lhsT=ones_sb, rhs=bu_sb[:, h * DM : (h + 1) * DM],
                        start=False, stop=True,
                    )
                nc.scalar.copy(out=o_sb[:, h * DM : (h + 1) * DM], in_=o_ps)
            nc.sync.dma_start(
                out=out[pv["oidx"] * P : (pv["oidx"] + 1) * P, :], in_=o_sb
            )

        prev = None   # block b-1 state
        prev2 = None  # block b-2 state
        for b in range(NB):
            is_out = b >= WARM

            xt_sb = sb_in.tile([P, D], bf16, tag="xt")
            nc.sync.dma_start(out=xt_sb, in_=xt[b])

            # ---- down-projection: hT[m, t] (4 chunks of 128 m) ----
            hT_ps = ps_hT.tile([P, MC, P], f32, tag="hT")
            for mc in range(MC):
                for dc in range(DC):
                    nc.tensor.matmul(
                        hT_ps[:, mc, :],
                        lhsT=wd_sb[:, dc, mc * P : (mc + 1) * P],
                        rhs=xt_sb[:, dc * P : (dc + 1) * P],
                        start=(dc == 0),
                        stop=(dc == DC - 1) if not use_bias else False,
                    )
                if use_bias:
                    nc.tensor.matmul(
                        hT_ps[:, mc, :],
                        lhsT=bd_sb[:, mc * P : (mc + 1) * P],
                        rhs=ones_sb,
                        start=False,
                        stop=True,
                    )
            hT_sb = sb_mid.tile([P, MC, P], bf16, tag="hT_sb")
            nc.scalar.copy(out=hT_sb, in_=hT_ps)

            # block b-2: up-projection matmuls + store (rT was copied to SBUF
            # last iteration; this also hides the hT copy before qkv)
            if prev2 is not None and prev2["oidx"] is not None:
                stage2_upmm(prev2)

            # block b-1: scan matmul, then r on DVE (overlaps with qkv below)
            if prev is not None:
                stage2_mem(prev)
                if prev["oidx"] is not None:
                    stage2_r(prev)

            # ---- k/v (+q for output blocks) projections ----
            k_ps = ps_big.tile([P, DM], f32, tag="qkv")
            v_ps = ps_big.tile([P, DM], f32, tag="qkv")
            if is_out:
                q_ps = ps_big.tile([P, DM], f32, tag="qkv")
            for mc in range(MC):
                last = mc == MC - 1
                nc.tensor.matmul(
                    k_ps, lhsT=hT_sb[:, mc], rhs=wk_sb[:, mc],
                    start=(mc == 0), stop=last and not use_bias,
                )
                nc.tensor.matmul(
                    v_ps, lhsT=hT_sb[:, mc], rhs=wv_sb[:, mc],
                    start=(mc == 0), stop=last and not use_bias,
                )
                if is_out:
                    nc.tensor.matmul(
                        q_ps, lhsT=hT_sb[:, mc], rhs=wq_sb[:, mc],
                        start=(mc == 0), stop=last and not use_bias,
                    )
            if use_bias:
                nc.tensor.matmul(k_ps, lhsT=ones_sb, rhs=bk_sb, start=False, stop=True)
                nc.tensor.matmul(v_ps, lhsT=ones_sb, rhs=bv_sb, start=False, stop=True)
                if is_out:
                    nc.tensor.matmul(
                        q_ps, lhsT=ones_sb, rhs=bq_sb, start=False, stop=True
                    )

            # block b-1: transposes (r computed on DVE during qkv above)
            if prev is not None and prev["oidx"] is not None:
                stage2_transp(prev)

            # ---- layernorms + u = k*v (+ carry fold into row 0) ----
            k_sb = layernorm(
                k_ps, bf16, "k",
                gtile=kng_sb if use_nb else None,
                btile=knb_sb if use_nb else None,
            )
            u_sb = sb_mid.tile([P, DM], bf16, tag="u")
            nc.vector.tensor_tensor(u_sb, k_sb, v_ps, OP.mult)
            if prev is not None:
                # u'[0] = u[0] + (gate/lr) * carry_prev
                nc.vector.scalar_tensor_tensor(
                    out=u_sb[0:1, :],
                    in0=prev["carry"],
                    scalar=gol_f,
                    in1=u_sb[0:1, :],
                    op0=OP.mult,
                    op1=OP.add,
                )
            if b < NB - 1:
                # outgoing carry = wcol^T @ u' -> SBUF for the next block
                c_ps = ps_cr.tile([1, DM], f32, tag="cps")
                nc.tensor.matmul(c_ps, lhsT=wcol_sb, rhs=u_sb, start=True, stop=True)
                carry_sb = sb_mid.tile([1, DM], bf16, tag="carry")
                nc.scalar.copy(out=carry_sb, in_=c_ps)
            else:
                carry_sb = None

            if is_out:
                q_sb = layernorm(
                    q_ps, bf16, "q",
                    gtile=qng_sb if use_nb else None,
                    btile=qnb_sb if use_nb else None,
                )
            else:
                q_sb = None

            prev2 = prev
            prev = {
                "q": q_sb,
                "u": u_sb,
                "carry": carry_sb,
                "oidx": (b - WARM) if is_out else None,
            }

        # epilogue: flush the pipeline
        if prev2 is not None and prev2["oidx"] is not None:
            stage2_upmm(prev2)
        if prev is not None and prev["oidx"] is not None:
            stage2_mem(prev)
            stage2_r(prev)
            stage2_transp(prev)
            stage2_upmm(prev)

    nc.compile()
    return nc


def prep_weights(Wd, bd, Wq, bq, Wk, bk, Wv, bv, Wu, bu,
                 qn_g, qn_b, kn_g, kn_b, adaptive_lr, forget_factor):
    """Host-side packing of all weight-derived inputs. Returns (in_map, flags)."""
    gate = float(1.0 - np.float64(forget_factor.reshape(-1)[0]))
    lr = float(np.float64(adaptive_lr.reshape(-1)[0]))

    use_bias = any(np.any(np.asarray(a) != 0) for a in (bd, bq, bk, bv, bu))
    use_nb = bool(np.any(np.asarray(qn_b) != 0) or np.any(np.asarray(kn_b) != 0))

    Wd = np.asarray(Wd, np.float32)
    Wq = np.asarray(Wq, np.float32)
    Wk = np.asarray(Wk, np.float32)
    Wv = np.asarray(Wv, np.float32).copy()
    Wu = np.asarray(Wu, np.float32).copy()
    bv_eff = np.asarray(bv, np.float32).copy()
    qn_g = np.asarray(qn_g, np.float32)
    kn_g = np.asarray(kn_g, np.float32)

    if not use_nb:
        # fold gains: k*g into v's weights (u = k*(g*v)), q*g into Wu rows
        Wv *= kn_g[:, None]
        bv_eff *= kn_g
        Wu *= qn_g[None, :]

    def pack_rhs(W, nchunks):  # W [N, K] -> [P, nchunks, N] with K on partitions
        A = np.ascontiguousarray(W.T)  # [K, N]
        return np.ascontiguousarray(
            A.reshape(nchunks, P, A.shape[1]).transpose(1, 0, 2)
        ).astype(BF16)

    wd_h = pack_rhs(Wd, DC)   # [128, 8, 512]  (used as lhsT tiles)
    wq_h = pack_rhs(Wq, MC)
    wk_h = pack_rhs(Wk, MC)
    wv_h = pack_rhs(Wv, MC)
    wu_h = pack_rhs(Wu, MC)   # [128, 4, 1024]

    ii = np.arange(P, dtype=np.float64)
    tt = ii[None, :]
    dd = tt - ii[:, None]  # t - i
    lt_h = np.where(dd >= 0, lr * gate ** np.maximum(dd, 0), 0.0).astype(BF16)
    wcol_h = (lr * gate ** (127.0 - ii)).reshape(P, 1).astype(BF16)
    ident_h = np.eye(P, dtype=np.float32).astype(BF16)

    in_map = {
        "wd": wd_h, "wq": wq_h, "wk": wk_h, "wv": wv_h, "wu": wu_h,
        "lt": lt_h, "wcol": wcol_h, "ident": ident_h,
    }
    if use_bias:
        in_map["bd"] = np.asarray(bd, np.float32).reshape(1, DM).astype(BF16)
        in_map["bq"] = np.asarray(bq, np.float32).reshape(1, DM).astype(BF16)
        in_map["bk"] = np.asarray(bk, np.float32).reshape(1, DM).astype(BF16)
        in_map["bv"] = bv_eff.reshape(1, DM).astype(BF16)
        in_map["bu"] = np.asarray(bu, np.float32).reshape(1, D).astype(BF16)
    if use_nb:
        in_map["qng"] = np.broadcast_to(qn_g, (P, DM)).astype(BF16)
        in_map["qnb"] = np.broadcast_to(np.asarray(qn_b, np.float32), (P, DM)).astype(BF16)
        in_map["kng"] = np.broadcast_to(kn_g, (P, DM)).astype(BF16)
        in_map["knb"] = np.broadcast_to(np.asarray(kn_b, np.float32), (P, DM)).astype(BF16)
    return in_map, use_bias, use_nb, float(gate / lr) if lr != 0.0 else 0.0


def pack_x_tokens(xs):
    """xs [T, D] f32 (T multiple of 128) -> [T//128, 128, 1024] bf16 tiled/transposed."""
    T = xs.shape[0]
    nb = T // P
    return np.ascontiguousarray(
        xs.reshape(nb, P, DC, P).transpose(0, 3, 2, 1).reshape(nb, P, D)
    ).astype(BF16)


_NC_CACHE = {}


def _get_nc(NB, WARM, n_cores, gol_f, use_bias, use_nb):
    key = (NB, WARM, n_cores, gol_f, use_bias, use_nb)
    if key not in _NC_CACHE:
        _NC_CACHE[key] = build_nc(NB, WARM, n_cores, gol_f, use_bias, use_nb)
    return _NC_CACHE[key]


def kernel(x, Wd, bd, Wq, bq, Wk, bk, Wv, bv, Wu, bu,
           qn_g, qn_b, kn_g, kn_b, adaptive_lr, forget_factor):
    global _LAST_RESULTS
    x = np.asarray(x, np.float32)
    assert x.shape == (B_FULL, S_FULL, D), x.shape

    w_map, use_bias, use_nb, gol_f = prep_weights(
        Wd, bd, Wq, bq, Wk, bk, Wv, bv, Wu, bu,
        qn_g, qn_b, kn_g, kn_b, adaptive_lr, forget_factor,
    )

    nc = _get_nc(NB_FULL, WARM_FULL, N_CORES, gol_f, use_bias, use_nb)

    half_len = S_FULL // 2  # 4096
    warm = WARM_FULL * P    # 256
    in_maps = []
    for c in range(N_CORES):
        bidx, half = divmod(c, 2)
        if half == 0:
            xs = np.zeros((warm + half_len, D), np.float32)
            xs[warm:] = x[bidx, :half_len]
        else:
            xs = x[bidx, half_len - warm :]
        m = dict(w_map)
        m["xt"] = pack_x_tokens(xs)
        in_maps.append(m)

    trace = bool(os.environ.get("KERNEL_TRACE"))
    res = bass_utils.run_bass_kernel_spmd(
        nc, in_maps, core_ids=list(range(N_CORES)), trace=trace
    )
    _LAST_RESULTS = res

    out = np.empty((B_FULL, S_FULL, D), np.float32)
    for c in range(N_CORES):
        bidx, half = divmod(c, 2)
        out[bidx, half * half_len : (half + 1) * half_len] = res.results[c]["out"]
    return out
